# revision 8
# baseline (speedup 1.0000x reference)
"""Causal self-attention with adapter prefix — Trainium2 Bass kernel.

Sharding: 8 cores = batch(2) x head-group(4). Each core computes 4 heads of
one batch element end-to-end and a [T, C] partial of the output projection
(W_proj column-slice); host sums the 4 partials per batch element.

All matmuls use float32r (fp32 @ ~bf16 speed on the PE, ~1.5e-4 rel err).
"""

import math
import numpy as np

B, T, C, H, HS, AL = 2, 2048, 2048, 16, 128, 10
HPC = 4           # heads per core
NCORES = 8
TC_ = 512         # t-chunk (matmul moving dim)
NTC = T // TC_    # 4
NCB = C // 128    # 16 contraction blocks
NEG = -30000.0
SCALE = 1.0 / math.sqrt(HS)

_cache = {}


def _build():
    import concourse.bacc as bacc
    import concourse.tile as tile
    import concourse.mybir as mybir

    f32 = mybir.dt.float32
    f32r = mybir.dt.float32r
    Exp = mybir.ActivationFunctionType.Exp

    nc = bacc.Bacc("TRN2", target_bir_lowering=False, debug=False,
                   num_devices=NCORES)

    XT = nc.dram_tensor("XT", [NCB, 128, T], f32r, kind="ExternalInput")
    WQKT = nc.dram_tensor("WQKT", [128, NCB, 1024], f32r, kind="ExternalInput")
    WVT = nc.dram_tensor("WVT", [128, NCB, 512], f32r, kind="ExternalInput")
    WPT = nc.dram_tensor("WPT", [128, 4, T], f32r, kind="ExternalInput")
    COSD = nc.dram_tensor("COSD", [128, T], f32, kind="ExternalInput")
    SINA = nc.dram_tensor("SINA", [128, T], f32, kind="ExternalInput")
    MASKS = nc.dram_tensor("MASKS", [128, 4, TC_], f32, kind="ExternalInput")
    AKT = nc.dram_tensor("AKT", [128, HPC * AL], f32r, kind="ExternalInput")
    AVC = nc.dram_tensor("AVC", [AL, HPC * HS], f32r, kind="ExternalInput")
    ONESC = nc.dram_tensor("ONESC", [128, 1], f32r, kind="ExternalInput")
    GAT = nc.dram_tensor("GAT", [1, HPC], f32, kind="ExternalInput")
    OUT = nc.dram_tensor("OUT", [T, C], f32, kind="ExternalOutput")

    with tile.TileContext(nc) as tc:
        with tc.tile_pool(name="dscr", bufs=1, space="DRAM") as dscr, \
             tc.tile_pool(name="dbnc", bufs=4, space="DRAM") as dbnc, \
             tc.tile_pool(name="ytp", bufs=1) as ytp:
            qk_scr = dscr.tile([1024, T], f32r)     # q heads 0-3 rows, then k
            qk_swp = dscr.tile([1024, T], f32r)     # pair-swapped rows (for RoPE)
            v_scr = dscr.tile([T, HPC * HS], f32r)  # [t, head*HS+d]
            yT = ytp.tile([128, HPC, T], f32r)      # per-head y^T (unnorm-combined)

            # ---------------- Phase 1: QKV projection ----------------
            with tc.tile_pool(name="p1w", bufs=1) as p1w, \
                 tc.tile_pool(name="p1x", bufs=2) as p1x, \
                 tc.tile_pool(name="p1s", bufs=4) as p1s, \
                 tc.tile_pool(name="p1ps", bufs=2, space="PSUM") as p1ps:
                wqk = p1w.tile([128, NCB, 1024], f32r)
                nc.sync.dma_start(out=wqk, in_=WQKT.ap())
                wv = p1w.tile([128, NCB, 512], f32r)
                nc.sync.dma_start(out=wv, in_=WVT.ap())

                for tci in range(NTC):
                    sl = slice(tci * TC_, (tci + 1) * TC_)
                    xt = p1x.tile([128, NCB, TC_], f32r, tag="xt")
                    nc.sync.dma_start(out=xt, in_=XT.ap()[:, :, sl].transpose([1, 0, 2]))
                    # q,k rows: out block ob covers o = ob*128..+128
                    for ob in range(8):
                        ps = p1ps.tile([128, TC_], f32, tag="ps")
                        for cb in range(NCB):
                            nc.tensor.matmul(ps, wqk[:, cb, ob * 128:(ob + 1) * 128],
                                             xt[:, cb, :],
                                             start=(cb == 0), stop=(cb == NCB - 1))
                        st = p1s.tile([128, TC_], f32r, tag="st")
                        nc.scalar.copy(st, ps)
                        nc.sync.dma_start(
                            out=qk_scr[ob * 128:(ob + 1) * 128, sl], in_=st)
                    # v rows: [t-block, 512]
                    for tb in range(4):
                        psv = p1ps.tile([128, 512], f32, tag="ps")
                        for cb in range(NCB):
                            nc.tensor.matmul(psv, xt[:, cb, tb * 128:(tb + 1) * 128],
                                             wv[:, cb, :],
                                             start=(cb == 0), stop=(cb == NCB - 1))
                        stv = p1s.tile([128, 512], f32r, tag="st")
                        nc.scalar.copy(stv, psv)
                        r0 = (tci * 4 + tb) * 128
                        nc.sync.dma_start(out=v_scr[r0:r0 + 128, :], in_=stv)

            # build pair-swapped copy of qk rows (DRAM->DRAM, linear APs)
            scr3 = qk_scr.rearrange("(a b) t -> a b t", b=2)
            swp3 = qk_swp.rearrange("(a b) t -> a b t", b=2)
            nc.sync.dma_start(out=swp3[:, 0, :], in_=scr3[:, 1, :])
            nc.sync.dma_start(out=swp3[:, 1, :], in_=scr3[:, 0, :])

            # ---------------- Phase 2: RoPE + SDPA + adapter ----------------
            with tc.tile_pool(name="p2c", bufs=1) as p2c, \
                 tc.tile_pool(name="p2r", bufs=2) as p2r, \
                 tc.tile_pool(name="p2k", bufs=2) as p2k, \
                 tc.tile_pool(name="p2p", bufs=3) as p2p, \
                 tc.tile_pool(name="p2v", bufs=3) as p2v, \
                 tc.tile_pool(name="p2m", bufs=2) as p2m, \
                 tc.tile_pool(name="p2ps", bufs=1, space="PSUM") as p2ps:
                cosd = p2c.tile([128, T], f32)
                nc.sync.dma_start(out=cosd, in_=COSD.ap())
                sina = p2c.tile([128, T], f32)
                nc.sync.dma_start(out=sina, in_=SINA.ap())
                maskt = p2c.tile([128, 4, TC_], f32)
                nc.sync.dma_start(out=maskt, in_=MASKS.ap())
                akt = p2c.tile([128, HPC * AL], f32r)
                nc.sync.dma_start(out=akt, in_=AKT.ap())
                avc = p2c.tile([AL, HPC * HS], f32r)
                nc.sync.dma_start(out=avc, in_=AVC.ap())
                onesc = p2c.tile([128, 1], f32r)
                nc.sync.dma_start(out=onesc, in_=ONESC.ap())
                gat = p2c.tile([1, HPC], f32)
                nc.sync.dma_start(out=gat, in_=GAT.ap())

                for h in range(HPC):
                    # --- RoPE for q and k of this head, chunk by chunk ---
                    rots = {}
                    for nm, row0 in (("q", h * 128), ("k", 512 + h * 128)):
                        rot = p2k.tile([128, T], f32r, tag=f"{nm}rot",
                                       name=f"{nm}rot_h{h}")
                        rows = qk_scr[row0:row0 + 128, :].bitcast(f32)
                        rows_sw = qk_swp[row0:row0 + 128, :].bitcast(f32)
                        for ch in range(NTC):
                            sl = slice(ch * TC_, (ch + 1) * TC_)
                            raw = p2r.tile([128, TC_], f32, tag="raw", name="raw")
                            nc.sync.dma_start(out=raw, in_=rows[:, sl])
                            swp = p2r.tile([128, TC_], f32, tag="swp", name="swp")
                            nc.sync.dma_start(out=swp, in_=rows_sw[:, sl])
                            t1 = p2r.tile([128, TC_], f32, tag="t1", name="t1")
                            nc.vector.tensor_mul(t1, raw, cosd[:, sl])
                            t2 = p2r.tile([128, TC_], f32, tag="t2", name="t2")
                            nc.vector.tensor_mul(t2, swp, sina[:, sl])
                            nc.vector.tensor_add(rot[:, sl], t1, t2)
                        rots[nm] = rot

                    # --- SDPA over i-chunks ---
                    for ic in range(NTC):
                        isl = slice(ic * TC_, (ic + 1) * TC_)
                        qrhs = rots["q"][:, isl]
                        njb = 4 * ic + 4
                        ps_y = p2ps.tile([128, TC_], f32, tag="y", bufs=2,
                                         name="ps_y")
                        ps_rs = p2ps.tile([1, TC_], f32, tag="rs", bufs=1,
                                          name="ps_rs")
                        for jb in range(njb):
                            ps_s = p2ps.tile([128, TC_], f32, tag="s", bufs=2,
                                             name="ps_s")
                            nc.tensor.matmul(ps_s,
                                             rots["k"][:, jb * 128:(jb + 1) * 128],
                                             qrhs, start=True, stop=True)
                            off = jb - 4 * ic
                            if off >= 0:
                                nc.vector.tensor_add(ps_s, ps_s, maskt[:, off, :])
                            pT = p2p.tile([128, TC_], f32r, tag="pT", name="pT")
                            nc.scalar.activation(pT, ps_s, Exp, scale=SCALE)
                            vt = p2v.tile([128, HS], f32r, tag="vt", name="vt")
                            nc.sync.dma_start(
                                out=vt,
                                in_=v_scr[jb * 128:(jb + 1) * 128,
                                          h * HS:(h + 1) * HS])
                            nc.tensor.matmul(ps_y, vt, pT,
                                             start=(jb == 0), stop=(jb == njb - 1))
                            nc.tensor.matmul(ps_rs, onesc, pT,
                                             start=(jb == 0), stop=(jb == njb - 1))

                        # --- adapter prefix attention for this i-chunk ---
                        ps_sa = p2ps.tile([AL, TC_], f32, tag="sa", bufs=1,
                                          name="ps_sa")
                        nc.tensor.matmul(ps_sa, akt[:, h * AL:(h + 1) * AL],
                                         qrhs, start=True, stop=True)
                        paT = p2p.tile([AL, TC_], f32r, tag="paT", name="paT")
                        nc.scalar.activation(paT, ps_sa, Exp, scale=SCALE)
                        ps_ay = p2ps.tile([128, TC_], f32, tag="ay", bufs=1,
                                          name="ps_ay")
                        nc.tensor.matmul(ps_ay, avc[:AL, h * HS:(h + 1) * HS],
                                         paT, start=True, stop=True)
                        ps_rsa = p2ps.tile([1, TC_], f32, tag="rsa", bufs=1,
                                           name="ps_rsa")
                        nc.tensor.matmul(ps_rsa, onesc[:AL, :], paT,
                                         start=True, stop=True)

                        # --- normalize + gated combine ---
                        rsrow = p2m.tile([1, TC_], f32, tag="rsrow", name="rsrow")
                        nc.scalar.copy(rsrow, ps_rs)
                        nc.vector.reciprocal(rsrow, rsrow)
                        rsarow = p2m.tile([1, TC_], f32, tag="rsarow", name="rsarow")
                        nc.scalar.copy(rsarow, ps_rsa)
                        nc.vector.reciprocal(rsarow, rsarow)
                        nc.vector.tensor_scalar_mul(rsarow, rsarow,
                                                    gat[0:1, h:h + 1])
                        rs_dr = dbnc.tile([1, TC_], f32, tag="rsdr", name="rs_dr")
                        nc.sync.dma_start(out=rs_dr, in_=rsrow)
                        rsa_dr = dbnc.tile([1, TC_], f32, tag="rsadr", name="rsa_dr")
                        nc.sync.dma_start(out=rsa_dr, in_=rsarow)
                        rb = p2m.tile([128, TC_], f32, tag="rb", name="rb")
                        nc.sync.dma_start(out=rb, in_=rs_dr.to_broadcast([128, TC_]))
                        rab = p2m.tile([128, TC_], f32, tag="rab", name="rab")
                        nc.sync.dma_start(out=rab, in_=rsa_dr.to_broadcast([128, TC_]))
                        tm1 = p2m.tile([128, TC_], f32, tag="tm1", name="tm1")
                        nc.vector.tensor_mul(tm1, ps_y, rb)
                        tm2 = p2m.tile([128, TC_], f32, tag="tm2", name="tm2")
                        nc.vector.tensor_mul(tm2, ps_ay, rab)
                        nc.vector.tensor_add(yT[:, h, isl], tm1, tm2)

            # ---------------- Phase 3: output projection ----------------
            with tc.tile_pool(name="p3w", bufs=1) as p3w, \
                 tc.tile_pool(name="p3s", bufs=4) as p3s, \
                 tc.tile_pool(name="p3ps", bufs=2, space="PSUM") as p3ps:
                wp = p3w.tile([128, 4, T], f32r)
                nc.sync.dma_start(out=wp, in_=WPT.ap())
                for tb in range(T // 128):
                    for oc in range(C // 512):
                        pso = p3ps.tile([128, 512], f32, tag="pso", name="pso")
                        for cib in range(4):
                            nc.tensor.matmul(
                                pso, yT[:, cib, tb * 128:(tb + 1) * 128],
                                wp[:, cib, oc * 512:(oc + 1) * 512],
                                start=(cib == 0), stop=(cib == 3))
                        stg = p3s.tile([128, 512], f32, tag="stg", name="stg")
                        nc.scalar.copy(stg, pso)
                        nc.sync.dma_start(
                            out=OUT.ap()[tb * 128:(tb + 1) * 128,
                                         oc * 512:(oc + 1) * 512], in_=stg)

    nc.compile()
    return nc


def _prep_core(x, ropec, W_attn, W_proj, adapter_emb, gating, b, hg):
    """Host-side input prep for core (b, hg)."""
    cos, sin = ropec
    xT = np.ascontiguousarray(x[b].T)                       # [C, T]
    XT = xT.reshape(NCB, 128, T)

    r0 = hg * 512
    Wq = W_attn[r0:r0 + 512]
    Wk = W_attn[C + r0:C + r0 + 512]
    Wv = W_attn[2 * C + r0:2 * C + r0 + 512]
    WQKT = np.ascontiguousarray(
        np.concatenate([Wq, Wk], 0).T.reshape(NCB, 128, 1024).transpose(1, 0, 2))
    WVT = np.ascontiguousarray(Wv.T.reshape(NCB, 128, 512).transpose(1, 0, 2))
    WPT = np.ascontiguousarray(
        W_proj[:, r0:r0 + 512].T.reshape(4, 128, T).transpose(1, 0, 2))

    cosD = np.repeat(cos.T, 2, axis=0)                      # [128, T]
    sinA = np.repeat(sin.T, 2, axis=0).copy()
    sinA[0::2] *= -1.0

    jl = np.arange(128)[:, None, None]
    off = np.arange(4)[None, :, None]
    il = np.arange(TC_)[None, None, :]
    MASKS_ = np.where(off * 128 + jl <= il, 0.0, NEG).astype(np.float32)

    ak = adapter_emb @ Wk.T                                 # [AL, 512]
    av = adapter_emb @ Wv.T                                 # [AL, 512]
    AKT = np.ascontiguousarray(
        ak.reshape(AL, HPC, HS).transpose(2, 1, 0).reshape(HS, HPC * AL))
    # AKT[d, h*AL+a] = ak[a, h*HS+d]
    AVC = np.ascontiguousarray(av)                          # [AL, h*HS+d]
    GATv = gating[0, hg * HPC:(hg + 1) * HPC, 0, 0].reshape(1, HPC)

    return {
        "XT": XT.astype(np.float32),
        "WQKT": WQKT.astype(np.float32),
        "WVT": WVT.astype(np.float32),
        "WPT": WPT.astype(np.float32),
        "COSD": cosD.astype(np.float32),
        "SINA": sinA.astype(np.float32),
        "MASKS": MASKS_.astype(np.float32),
        "AKT": AKT.astype(np.float32),
        "AVC": AVC.astype(np.float32),
        "ONESC": np.ones((128, 1), np.float32),
        "GAT": GATv.astype(np.float32),
    }


def kernel(x, rope, mask, W_attn, W_proj, adapter_emb, gating,
           max_seq_length=None, **_):
    from concourse.bass_utils import run_bass_kernel_spmd

    x = np.asarray(x, np.float32)
    rope = np.asarray(rope, np.float32)
    W_attn = np.asarray(W_attn, np.float32)
    W_proj = np.asarray(W_proj, np.float32)
    adapter_emb = np.asarray(adapter_emb, np.float32)
    gating = np.asarray(gating, np.float32)

    if "nc" not in _cache:
        _cache["nc"] = _build()
    nc = _cache["nc"]

    ropec = (np.ascontiguousarray(rope[:, :, 0]), np.ascontiguousarray(rope[:, :, 1]))
    in_maps = [
        _prep_core(x, ropec, W_attn, W_proj, adapter_emb, gating, b, hg)
        for b in range(B) for hg in range(4)
    ]
    _cache["in_maps"] = in_maps
    res = run_bass_kernel_spmd(nc, in_maps, core_ids=list(range(NCORES)))
    outs = [r["OUT"] for r in res.results]
    y = np.empty((B, T, C), np.float32)
    for b in range(B):
        y[b] = outs[4 * b] + outs[4 * b + 1] + outs[4 * b + 2] + outs[4 * b + 3]
    return y
